# revision 14
# baseline (speedup 1.0000x reference)
"""Masked Hillis-Steele scan kernel for Trainium2 (8 NeuronCores, SPMD).

Problem: B=131072 rows, L=512. For each row:
    y = where(mask, x, 0)
    for s in [1,2,4,...,512]:  # s=512 step is a no-op (shift of full row)
        y[i] += y[i-s]  if mask[i] and mask[i-s]

Key algebraic fact: unmasked positions of y stay 0 forever, so
    mask[i-s]*y[i-s] == y[i-s]  and each step is  y += mask * shift_s(y).

Hybrid three-engine design, fp16 on-chip, x pre-masked on the host:

* PE path (most rows, transposed layout [L on partitions as 4 banks of
  128, batch on free]): the TensorEngine accumulates w += shift_s(p_k)
  in PSUM via eye-matrix matmuls (within-bank eye(k=s) + cross-bank
  eye(k=s-128), fp16 single-pass, exact for 0/1 matrices, one matmul
  per PSUM bank — moving free size is capped at 512). The Act engine
  drains w to SBUF with an f32->fp16 cast, and the DVE computes
  p_{k+1} = mask * w in the 2x_1p perf mode (packed fp16, all SBUF).
  p_9 is the output. Supertile chains are emitted in step-interleaved
  PAIRS (PSUM fits two) so the in-order engine queues always hold
  ready work from the sibling chain.

* Flat path (remaining rows, [rows on partitions, L on free]): shift is
  a free-dim AP offset; per step one TensorTensor mul + add at 2x_1p.
  Interleaved between PE steps to fill the DVE's drain-wait gaps.

Mask is DMA'd as uint8 (1/4 the bytes) and cast to fp16 on Act.

Sharding: pure data parallel over B across the 8 cores.
"""

import os
import sys

import numpy as np

sys.path.insert(0, "/opt/trn_rl_repo")

B = 131072
L = 512
N_CORES = 8
BP = B // N_CORES  # rows per core = 16384

SHIFTS = [1, 2, 4, 8, 16, 32, 64, 128, 256]

NB = 512  # batch columns per PE supertile
N_SUP_PE = 20  # PE-path supertiles per core -> 10240 rows
PE_ROWS = N_SUP_PE * NB
FLAT_ROWS = BP - PE_ROWS  # 6144
GF = 8  # flat row-groups per tile
FLAT_TILE_ROWS = 128 * GF  # 1024
N_FLAT_TILES = FLAT_ROWS // FLAT_TILE_ROWS  # 6
assert FLAT_ROWS % FLAT_TILE_ROWS == 0 and N_SUP_PE % 2 == 0

# stationary eye matrices: within-bank k=s, cross-bank k=s-128, eye0
EYE_KS = [1, 2, 4, 8, 16, 32, 64] + [s - 128 for s in [1, 2, 4, 8, 16, 32, 64]] + [0]
EYE_IDX = {k: i for i, k in enumerate(EYE_KS)}
N_EYES = len(EYE_KS)  # 15

_last_results = None  # stash for test harness introspection


def _eye_mats():
    """[128, N_EYES, 128] fp16: mats[:, i, :] = eye(128, k=EYE_KS[i]) (lhsT)."""
    m = np.stack([np.eye(128, k=k, dtype=np.float16) for k in EYE_KS])
    return np.ascontiguousarray(m.transpose(1, 0, 2))


def _legalize_waits(nc, cap=1):
    """Walrus's TRN2 instruction encodings only have room for a small number
    of sync-wait commands (1 for DMA/3D-AP tensor ops); Tile freely attaches
    more. Hoist surplus waits into standalone event-semaphore (wait-only)
    instructions inserted just before the over-subscribed instruction on the
    same engine queue."""
    import concourse.mybir as mybir

    n_new = 0
    for f in nc.m.functions:
        for b in f.blocks:
            new_list = []
            for ins in b.instructions:
                si = ins.sync_info
                if si is not None and len(si.on_wait) > cap:
                    waits = list(si.on_wait)
                    extra, keep = waits[:-cap], waits[-cap:]
                    for w in extra:
                        ev = mybir.InstEventSemaphore(
                            name=f"waitsplit_{n_new}", ins=[], outs=[]
                        )
                        ev.engine = ins.engine
                        ev.sync_info = mybir.SyncInfo(on_wait=[w], on_update=[])
                        new_list.append(ev)
                        n_new += 1
                    ins.sync_info = mybir.SyncInfo(
                        on_wait=keep, on_update=list(si.on_update)
                    )
                new_list.append(ins)
            b.instructions[:] = new_list
    return n_new


def _drain_banks(s):
    """PSUM bank range whose l >= s, rounded down to whole banks (recomputing
    p at l < s is a no-op; partial-partition PSUM APs spanning > 32
    partitions are rejected by the BIR verifier)."""
    if s < 128:
        return slice(0, 4)
    if s == 128:
        return slice(1, 4)
    return slice(2, 4)


def _build_hybrid_program(reps=1, legalize=True):
    from contextlib import ExitStack

    import concourse.bass as bass
    import concourse.mybir as mybir
    import concourse.tile as tile

    f16 = mybir.dt.float16
    f32 = mybir.dt.float32
    u8 = mybir.dt.uint8

    nc = bass.Bass(target_bir_lowering=False, debug=False)
    xT_ext = nc.declare_dram_parameter("xT", [L, PE_ROWS], f16, isOutput=False)
    mT_ext = nc.declare_dram_parameter("mT", [L, PE_ROWS], u8, isOutput=False)
    yT_ext = nc.declare_dram_parameter("yT", [L, PE_ROWS], f16, isOutput=True)
    mats_ext = nc.declare_dram_parameter("mats", [128, N_EYES, 128], f16, isOutput=False)
    xf_ext = nc.declare_dram_parameter("xf", [FLAT_ROWS, L], f16, isOutput=False)
    mf_ext = nc.declare_dram_parameter("mf", [FLAT_ROWS, L], u8, isOutput=False)
    yf_ext = nc.declare_dram_parameter("yf", [FLAT_ROWS, L], f16, isOutput=True)

    with tile.TileContext(nc) as tc, ExitStack() as ctx:
        cp = ctx.enter_context(tc.tile_pool(name="cp", bufs=1))
        pp = ctx.enter_context(tc.tile_pool(name="pp", bufs=3))
        m8p = ctx.enter_context(tc.tile_pool(name="m8p", bufs=3))
        mpp = ctx.enter_context(tc.tile_pool(name="mpp", bufs=3))
        dp = ctx.enter_context(tc.tile_pool(name="dp", bufs=4))
        wp = ctx.enter_context(tc.tile_pool(name="wp", bufs=2, space="PSUM"))
        xfp = ctx.enter_context(tc.tile_pool(name="xfp", bufs=3))
        mf8p = ctx.enter_context(tc.tile_pool(name="mf8p", bufs=3))
        mfp = ctx.enter_context(tc.tile_pool(name="mfp", bufs=3))
        tfp = ctx.enter_context(tc.tile_pool(name="tfp", bufs=2))

        mats_t = cp.tile([128, N_EYES, 128], f16)
        nc.sync.dma_start(mats_t[:], mats_ext[:])

        def eye(k):
            return mats_t[:, EYE_IDX[k], :]

        def flat_ops():
            """Generator: yields after each schedulable unit of flat work."""
            for r in range(N_FLAT_TILES):
                rows = slice(r * FLAT_TILE_ROWS, (r + 1) * FLAT_TILE_ROWS)
                xt = xfp.tile([128, GF, L], f16)
                m8 = mf8p.tile([128, GF, L], u8)
                mt = mfp.tile([128, GF, L], f16)
                tt = tfp.tile([128, GF, L], f16)
                nc.sync.dma_start(
                    xt[:], xf_ext[rows, :].rearrange("(g p) l -> p g l", p=128)
                )
                nc.sync.dma_start(
                    m8[:], mf_ext[rows, :].rearrange("(g p) l -> p g l", p=128)
                )
                nc.scalar.copy(mt[:], m8[:])
                yield
                for s in SHIFTS:
                    nc.vector.tensor_mul(
                        tt[:, :, s:], xt[:, :, : L - s], mt[:, :, s:]
                    )
                    nc.vector.tensor_add(
                        xt[:, :, s:], xt[:, :, s:], tt[:, :, s:]
                    )
                    yield
                nc.sync.dma_start(
                    yf_ext[rows, :].rearrange("(g p) l -> p g l", p=128), xt[:]
                )
                yield

        def emit_mms(wt, pt, s, is_last):
            """shift-s accumulate: w += shift_s(p). One matmul per PSUM bank
            (a single matmul's moving free size is capped at 512 = 1 bank)."""
            if s < 128:
                for b in range(4):
                    nc.tensor.matmul(
                        wt[:, b, :], eye(s), pt[:, b, :],
                        start=False, stop=False, skip_group_check=True,
                    )
                for b in range(1, 4):
                    nc.tensor.matmul(
                        wt[:, b, :], eye(s - 128), pt[:, b - 1, :],
                        start=False, stop=False, skip_group_check=True,
                    )
            elif s == 128:
                for b in range(1, 4):
                    nc.tensor.matmul(
                        wt[:, b, :], eye(0), pt[:, b - 1, :],
                        start=False, stop=False, skip_group_check=True,
                    )
            else:
                for b in range(2, 4):
                    nc.tensor.matmul(
                        wt[:, b, :], eye(0), pt[:, b - 2, :],
                        start=False, stop=is_last, skip_group_check=True,
                    )

        def body(_iv=None):
            fgen = flat_ops()
            n_units = N_FLAT_TILES * (len(SHIFTS) + 2)
            n_slots = (N_SUP_PE // 2) * (len(SHIFTS) + 1)
            credit, fdone = 0.0, False
            rate = n_units / n_slots

            def pump():
                nonlocal credit, fdone
                credit += rate
                while credit >= 1.0 and not fdone:
                    credit -= 1.0
                    try:
                        next(fgen)
                    except StopIteration:
                        fdone = True

            for jp in range(N_SUP_PE // 2):
                pair = (2 * jp, 2 * jp + 1)
                pts, mts, wts = [], [], []
                for j in pair:
                    cols = slice(j * NB, (j + 1) * NB)
                    pt = pp.tile([128, 4, NB], f16)  # p_k, updated in place
                    m8t = m8p.tile([128, 4, NB], u8)
                    mt = mpp.tile([128, 4, NB], f16)
                    nc.sync.dma_start(
                        pt[:],
                        xT_ext[:, cols].rearrange("(blk p) b -> p blk b", p=128),
                    )
                    nc.sync.dma_start(
                        m8t[:],
                        mT_ext[:, cols].rearrange("(blk p) b -> p blk b", p=128),
                    )
                    nc.scalar.copy(mt[:], m8t[:])
                    pts.append(pt)
                    mts.append(mt)
                for i in range(2):
                    wt = wp.tile([128, 4, NB], f32)
                    for b in range(4):
                        nc.tensor.matmul(
                            wt[:, b, :], eye(0), pts[i][:, b, :],
                            start=True, stop=False, skip_group_check=True,
                        )
                    wts.append(wt)
                pump()
                for s in SHIFTS:
                    is_last = s == SHIFTS[-1]
                    for i in range(2):
                        emit_mms(wts[i], pts[i], s, is_last)
                    dts = []
                    for i in range(2):
                        dt = dp.tile([128, 4, NB], f16)
                        bs = _drain_banks(s)
                        nc.scalar.copy(dt[:, bs, :], wts[i][:, bs, :])
                        dts.append(dt)
                    for i in range(2):
                        bs = _drain_banks(s)
                        nc.vector.tensor_mul(
                            pts[i][:, bs, :], dts[i][:, bs, :], mts[i][:, bs, :]
                        )
                    pump()
                for i, j in enumerate(pair):
                    cols = slice(j * NB, (j + 1) * NB)
                    nc.sync.dma_start(
                        yT_ext[:, cols].rearrange("(blk p) b -> p blk b", p=128),
                        pts[i][:],
                    )

            while not fdone:
                try:
                    next(fgen)
                except StopIteration:
                    fdone = True

        if reps == 1:
            body()
        else:
            with tc.For_i(0, reps, 1) as iv:
                body(iv)

    if legalize:
        _legalize_waits(nc)
    return nc


_cached = {}


def kernel(x, mask):
    global _last_results
    from concourse.bass_utils import run_bass_kernel_spmd

    x = np.asarray(x)
    m = np.asarray(mask)
    assert x.shape == (B, L) and m.shape == (B, L)
    # Host pre-masking: p0 = where(mask, x, 0)
    x16 = np.where(m, x, np.float32(0.0)).astype(np.float16)
    m8 = m.astype(np.uint8)

    if "hybrid" not in _cached:
        _cached["hybrid"] = _build_hybrid_program()
    nc = _cached["hybrid"]

    core_ids = list(range(N_CORES))
    mats = _eye_mats()
    in_maps = []
    for i in core_ids:
        xc = x16[i * BP : (i + 1) * BP]
        mc = m8[i * BP : (i + 1) * BP]
        in_maps.append(
            {
                "xf": np.ascontiguousarray(xc[:FLAT_ROWS]),
                "mf": np.ascontiguousarray(mc[:FLAT_ROWS]),
                "xT": np.ascontiguousarray(xc[FLAT_ROWS:].T),
                "mT": np.ascontiguousarray(mc[FLAT_ROWS:].T),
                "mats": mats,
            }
        )

    res = run_bass_kernel_spmd(nc, in_maps, core_ids)
    _last_results = res

    out = np.empty((B, L), dtype=np.float32)
    for i in core_ids:
        out[i * BP : i * BP + FLAT_ROWS] = res.results[i]["yf"].astype(np.float32)
        out[i * BP + FLAT_ROWS : (i + 1) * BP] = (
            res.results[i]["yT"].T.astype(np.float32)
        )
    return out


# revision 15
# speedup vs baseline: 1.0694x; 1.0694x over previous
"""Masked Hillis-Steele scan kernel for Trainium2 (8 NeuronCores, SPMD).

Problem: B=131072 rows, L=512. For each row:
    y = where(mask, x, 0)
    for s in [1,2,4,...,512]:  # s=512 step is a no-op (shift of full row)
        y[i] += y[i-s]  if mask[i] and mask[i-s]

Key algebraic fact: unmasked positions of y stay 0 forever, so
    mask[i-s]*y[i-s] == y[i-s]  and each step is  y += mask * shift_s(y).

Hybrid three-engine design, fp16 on-chip, x pre-masked on the host:

* PE path (most rows, transposed layout [L on partitions as 4 banks of
  128, batch on free]): the TensorEngine accumulates w += shift_s(p_k)
  in PSUM via eye-matrix matmuls (within-bank eye(k=s) + cross-bank
  eye(k=s-128), fp16 single-pass, exact for 0/1 matrices, one matmul
  per PSUM bank — moving free size is capped at 512). The Act engine
  drains w to SBUF with an f32->fp16 cast, and the DVE computes
  p_{k+1} = mask * w in the 2x_1p perf mode (packed fp16, all SBUF).
  p_9 is the output. Supertile chains are emitted in step-interleaved
  PAIRS (PSUM fits two) so the in-order engine queues always hold
  ready work from the sibling chain.

* Flat path (remaining rows, [rows on partitions, L on free]): shift is
  a free-dim AP offset; per step one TensorTensor mul + add at 2x_1p.
  Interleaved between PE steps to fill the DVE's drain-wait gaps.

Mask is DMA'd as uint8 (1/4 the bytes) and cast to fp16 on Act.

Sharding: pure data parallel over B across the 8 cores.
"""

import os
import sys

import numpy as np

sys.path.insert(0, "/opt/trn_rl_repo")

B = 131072
L = 512
N_CORES = 8
BP = B // N_CORES  # rows per core = 16384

SHIFTS = [1, 2, 4, 8, 16, 32, 64, 128, 256]

NB = 512  # batch columns per PE supertile
N_SUP_PE = 20  # PE-path supertiles per core -> 10240 rows
PE_ROWS = N_SUP_PE * NB
FLAT_ROWS = BP - PE_ROWS  # 6144
GF = 4  # flat row-groups per tile (small: keeps DVE queue slots short so
# PE-chain muls aren't head-of-line blocked behind long flat ops)
FLAT_TILE_ROWS = 128 * GF  # 512
N_FLAT_TILES = FLAT_ROWS // FLAT_TILE_ROWS  # 12
assert FLAT_ROWS % FLAT_TILE_ROWS == 0 and N_SUP_PE % 2 == 0

# stationary eye matrices: within-bank k=s, cross-bank k=s-128, eye0
EYE_KS = [1, 2, 4, 8, 16, 32, 64] + [s - 128 for s in [1, 2, 4, 8, 16, 32, 64]] + [0]
EYE_IDX = {k: i for i, k in enumerate(EYE_KS)}
N_EYES = len(EYE_KS)  # 15

_last_results = None  # stash for test harness introspection


def _eye_mats():
    """[128, N_EYES, 128] fp16: mats[:, i, :] = eye(128, k=EYE_KS[i]) (lhsT)."""
    m = np.stack([np.eye(128, k=k, dtype=np.float16) for k in EYE_KS])
    return np.ascontiguousarray(m.transpose(1, 0, 2))


def _legalize_waits(nc, cap=1):
    """Walrus's TRN2 instruction encodings only have room for a small number
    of sync-wait commands (1 for DMA/3D-AP tensor ops); Tile freely attaches
    more. Hoist surplus waits into standalone event-semaphore (wait-only)
    instructions inserted just before the over-subscribed instruction on the
    same engine queue."""
    import concourse.mybir as mybir

    n_new = 0
    for f in nc.m.functions:
        for b in f.blocks:
            new_list = []
            for ins in b.instructions:
                si = ins.sync_info
                if si is not None and len(si.on_wait) > cap:
                    waits = list(si.on_wait)
                    extra, keep = waits[:-cap], waits[-cap:]
                    for w in extra:
                        ev = mybir.InstEventSemaphore(
                            name=f"waitsplit_{n_new}", ins=[], outs=[]
                        )
                        ev.engine = ins.engine
                        ev.sync_info = mybir.SyncInfo(on_wait=[w], on_update=[])
                        new_list.append(ev)
                        n_new += 1
                    ins.sync_info = mybir.SyncInfo(
                        on_wait=keep, on_update=list(si.on_update)
                    )
                new_list.append(ins)
            b.instructions[:] = new_list
    return n_new


def _drain_banks(s):
    """PSUM bank range whose l >= s, rounded down to whole banks (recomputing
    p at l < s is a no-op; partial-partition PSUM APs spanning > 32
    partitions are rejected by the BIR verifier)."""
    if s < 128:
        return slice(0, 4)
    if s == 128:
        return slice(1, 4)
    return slice(2, 4)


def _build_hybrid_program(reps=1, legalize=True):
    from contextlib import ExitStack

    import concourse.bass as bass
    import concourse.mybir as mybir
    import concourse.tile as tile

    f16 = mybir.dt.float16
    f32 = mybir.dt.float32
    u8 = mybir.dt.uint8

    nc = bass.Bass(target_bir_lowering=False, debug=False)
    xT_ext = nc.declare_dram_parameter("xT", [L, PE_ROWS], f16, isOutput=False)
    mT_ext = nc.declare_dram_parameter("mT", [L, PE_ROWS], u8, isOutput=False)
    yT_ext = nc.declare_dram_parameter("yT", [L, PE_ROWS], f16, isOutput=True)
    mats_ext = nc.declare_dram_parameter("mats", [128, N_EYES, 128], f16, isOutput=False)
    xf_ext = nc.declare_dram_parameter("xf", [FLAT_ROWS, L], f16, isOutput=False)
    mf_ext = nc.declare_dram_parameter("mf", [FLAT_ROWS, L], u8, isOutput=False)
    yf_ext = nc.declare_dram_parameter("yf", [FLAT_ROWS, L], f16, isOutput=True)

    with tile.TileContext(nc) as tc, ExitStack() as ctx:
        cp = ctx.enter_context(tc.tile_pool(name="cp", bufs=1))
        pp = ctx.enter_context(tc.tile_pool(name="pp", bufs=3))
        m8p = ctx.enter_context(tc.tile_pool(name="m8p", bufs=3))
        mpp = ctx.enter_context(tc.tile_pool(name="mpp", bufs=3))
        dp = ctx.enter_context(tc.tile_pool(name="dp", bufs=4))
        wp = ctx.enter_context(tc.tile_pool(name="wp", bufs=2, space="PSUM"))
        xfp = ctx.enter_context(tc.tile_pool(name="xfp", bufs=3))
        mf8p = ctx.enter_context(tc.tile_pool(name="mf8p", bufs=3))
        mfp = ctx.enter_context(tc.tile_pool(name="mfp", bufs=3))
        tfp = ctx.enter_context(tc.tile_pool(name="tfp", bufs=2))

        mats_t = cp.tile([128, N_EYES, 128], f16)
        nc.sync.dma_start(mats_t[:], mats_ext[:])

        def eye(k):
            return mats_t[:, EYE_IDX[k], :]

        def flat_ops():
            """Generator: yields after each schedulable unit of flat work."""
            for r in range(N_FLAT_TILES):
                rows = slice(r * FLAT_TILE_ROWS, (r + 1) * FLAT_TILE_ROWS)
                xt = xfp.tile([128, GF, L], f16)
                m8 = mf8p.tile([128, GF, L], u8)
                mt = mfp.tile([128, GF, L], f16)
                tt = tfp.tile([128, GF, L], f16)
                nc.sync.dma_start(
                    xt[:], xf_ext[rows, :].rearrange("(g p) l -> p g l", p=128)
                )
                nc.sync.dma_start(
                    m8[:], mf_ext[rows, :].rearrange("(g p) l -> p g l", p=128)
                )
                nc.scalar.copy(mt[:], m8[:])
                yield
                for s in SHIFTS:
                    nc.vector.tensor_mul(
                        tt[:, :, s:], xt[:, :, : L - s], mt[:, :, s:]
                    )
                    nc.vector.tensor_add(
                        xt[:, :, s:], xt[:, :, s:], tt[:, :, s:]
                    )
                    yield
                nc.sync.dma_start(
                    yf_ext[rows, :].rearrange("(g p) l -> p g l", p=128), xt[:]
                )
                yield

        def emit_mms(wt, pt, s, is_last):
            """shift-s accumulate: w += shift_s(p). One matmul per PSUM bank
            (a single matmul's moving free size is capped at 512 = 1 bank)."""
            if s < 128:
                for b in range(4):
                    nc.tensor.matmul(
                        wt[:, b, :], eye(s), pt[:, b, :],
                        start=False, stop=False, skip_group_check=True,
                    )
                for b in range(1, 4):
                    nc.tensor.matmul(
                        wt[:, b, :], eye(s - 128), pt[:, b - 1, :],
                        start=False, stop=False, skip_group_check=True,
                    )
            elif s == 128:
                for b in range(1, 4):
                    nc.tensor.matmul(
                        wt[:, b, :], eye(0), pt[:, b - 1, :],
                        start=False, stop=False, skip_group_check=True,
                    )
            else:
                for b in range(2, 4):
                    nc.tensor.matmul(
                        wt[:, b, :], eye(0), pt[:, b - 2, :],
                        start=False, stop=is_last, skip_group_check=True,
                    )

        def body(_iv=None):
            fgen = flat_ops()
            n_units = N_FLAT_TILES * (len(SHIFTS) + 2)
            n_slots = (N_SUP_PE // 2) * (len(SHIFTS) + 1)
            credit, fdone = 0.0, False
            rate = n_units / n_slots

            def pump():
                nonlocal credit, fdone
                credit += rate
                while credit >= 1.0 and not fdone:
                    credit -= 1.0
                    try:
                        next(fgen)
                    except StopIteration:
                        fdone = True

            for jp in range(N_SUP_PE // 2):
                pair = (2 * jp, 2 * jp + 1)
                pts, mts, wts = [], [], []
                for j in pair:
                    cols = slice(j * NB, (j + 1) * NB)
                    pt = pp.tile([128, 4, NB], f16)  # p_k, updated in place
                    m8t = m8p.tile([128, 4, NB], u8)
                    mt = mpp.tile([128, 4, NB], f16)
                    nc.sync.dma_start(
                        pt[:],
                        xT_ext[:, cols].rearrange("(blk p) b -> p blk b", p=128),
                    )
                    nc.sync.dma_start(
                        m8t[:],
                        mT_ext[:, cols].rearrange("(blk p) b -> p blk b", p=128),
                    )
                    nc.scalar.copy(mt[:], m8t[:])
                    pts.append(pt)
                    mts.append(mt)
                for i in range(2):
                    wt = wp.tile([128, 4, NB], f32)
                    for b in range(4):
                        nc.tensor.matmul(
                            wt[:, b, :], eye(0), pts[i][:, b, :],
                            start=True, stop=False, skip_group_check=True,
                        )
                    wts.append(wt)
                pump()
                for s in SHIFTS:
                    is_last = s == SHIFTS[-1]
                    for i in range(2):
                        emit_mms(wts[i], pts[i], s, is_last)
                    dts = []
                    for i in range(2):
                        dt = dp.tile([128, 4, NB], f16)
                        bs = _drain_banks(s)
                        nc.scalar.copy(dt[:, bs, :], wts[i][:, bs, :])
                        dts.append(dt)
                    for i in range(2):
                        bs = _drain_banks(s)
                        nc.vector.tensor_mul(
                            pts[i][:, bs, :], dts[i][:, bs, :], mts[i][:, bs, :]
                        )
                    pump()
                for i, j in enumerate(pair):
                    cols = slice(j * NB, (j + 1) * NB)
                    nc.sync.dma_start(
                        yT_ext[:, cols].rearrange("(blk p) b -> p blk b", p=128),
                        pts[i][:],
                    )

            while not fdone:
                try:
                    next(fgen)
                except StopIteration:
                    fdone = True

        if reps == 1:
            body()
        else:
            with tc.For_i(0, reps, 1) as iv:
                body(iv)

    if legalize:
        _legalize_waits(nc)
    return nc


_cached = {}


def kernel(x, mask):
    global _last_results
    from concourse.bass_utils import run_bass_kernel_spmd

    x = np.asarray(x)
    m = np.asarray(mask)
    assert x.shape == (B, L) and m.shape == (B, L)
    # Host pre-masking: p0 = where(mask, x, 0)
    x16 = np.where(m, x, np.float32(0.0)).astype(np.float16)
    m8 = m.astype(np.uint8)

    if "hybrid" not in _cached:
        _cached["hybrid"] = _build_hybrid_program()
    nc = _cached["hybrid"]

    core_ids = list(range(N_CORES))
    mats = _eye_mats()
    in_maps = []
    for i in core_ids:
        xc = x16[i * BP : (i + 1) * BP]
        mc = m8[i * BP : (i + 1) * BP]
        in_maps.append(
            {
                "xf": np.ascontiguousarray(xc[:FLAT_ROWS]),
                "mf": np.ascontiguousarray(mc[:FLAT_ROWS]),
                "xT": np.ascontiguousarray(xc[FLAT_ROWS:].T),
                "mT": np.ascontiguousarray(mc[FLAT_ROWS:].T),
                "mats": mats,
            }
        )

    res = run_bass_kernel_spmd(nc, in_maps, core_ids)
    _last_results = res

    out = np.empty((B, L), dtype=np.float32)
    for i in core_ids:
        out[i * BP : i * BP + FLAT_ROWS] = res.results[i]["yf"].astype(np.float32)
        out[i * BP + FLAT_ROWS : (i + 1) * BP] = (
            res.results[i]["yT"].T.astype(np.float32)
        )
    return out


# revision 17
# speedup vs baseline: 1.0755x; 1.0057x over previous
"""Masked Hillis-Steele scan kernel for Trainium2 (8 NeuronCores, SPMD).

Problem: B=131072 rows, L=512. For each row:
    y = where(mask, x, 0)
    for s in [1,2,4,...,512]:  # s=512 step is a no-op (shift of full row)
        y[i] += y[i-s]  if mask[i] and mask[i-s]

Key algebraic fact: unmasked positions of y stay 0 forever, so
    mask[i-s]*y[i-s] == y[i-s]  and each step is  y += mask * shift_s(y).

Hybrid three-engine design, fp16 on-chip, x pre-masked on the host:

* PE path (most rows, transposed layout [L on partitions as 4 banks of
  128, batch on free]): the TensorEngine accumulates w += shift_s(p_k)
  in PSUM via eye-matrix matmuls (within-bank eye(k=s) + cross-bank
  eye(k=s-128), fp16 single-pass, exact for 0/1 matrices, one matmul
  per PSUM bank — moving free size is capped at 512). The Act engine
  drains w to SBUF with an f32->fp16 cast, and the DVE computes
  p_{k+1} = mask * w in the 2x_1p perf mode (packed fp16, all SBUF).
  p_9 is the output. Supertile chains are emitted in step-interleaved
  PAIRS (PSUM fits two) so the in-order engine queues always hold
  ready work from the sibling chain.

* Flat path (remaining rows, [rows on partitions, L on free]): shift is
  a free-dim AP offset; per step one TensorTensor mul + add at 2x_1p.
  Interleaved between PE steps to fill the DVE's drain-wait gaps.

Mask is DMA'd as uint8 (1/4 the bytes) and cast to fp16 on Act.

Sharding: pure data parallel over B across the 8 cores.
"""

import os
import sys

import numpy as np

sys.path.insert(0, "/opt/trn_rl_repo")

B = 131072
L = 512
N_CORES = 8
BP = B // N_CORES  # rows per core = 16384

SHIFTS = [1, 2, 4, 8, 16, 32, 64, 128, 256]

NB = 512  # batch columns per PE supertile
N_SUP_PE = 20  # PE-path supertiles per core -> 10240 rows
PE_ROWS = N_SUP_PE * NB
FLAT_ROWS = BP - PE_ROWS  # 6144
GF = 4  # flat row-groups per tile (small: keeps DVE queue slots short so
# PE-chain muls aren't head-of-line blocked behind long flat ops)
FLAT_TILE_ROWS = 128 * GF  # 512
N_FLAT_TILES = FLAT_ROWS // FLAT_TILE_ROWS  # 12
assert FLAT_ROWS % FLAT_TILE_ROWS == 0 and N_SUP_PE % 2 == 0

# stationary eye matrices: within-bank k=s, cross-bank k=s-128, eye0
EYE_KS = [1, 2, 4, 8, 16, 32, 64] + [s - 128 for s in [1, 2, 4, 8, 16, 32, 64]] + [0]
EYE_IDX = {k: i for i, k in enumerate(EYE_KS)}
N_EYES = len(EYE_KS)  # 15

_last_results = None  # stash for test harness introspection


def _eye_mats():
    """[128, N_EYES, 128] fp16: mats[:, i, :] = eye(128, k=EYE_KS[i]) (lhsT)."""
    m = np.stack([np.eye(128, k=k, dtype=np.float16) for k in EYE_KS])
    return np.ascontiguousarray(m.transpose(1, 0, 2))


def _legalize_waits(nc, cap=1):
    """Walrus's TRN2 instruction encodings only have room for a small number
    of sync-wait commands (1 for DMA/3D-AP tensor ops); Tile freely attaches
    more. Hoist surplus waits into standalone event-semaphore (wait-only)
    instructions inserted just before the over-subscribed instruction on the
    same engine queue."""
    import concourse.mybir as mybir

    n_new = 0
    for f in nc.m.functions:
        for b in f.blocks:
            new_list = []
            for ins in b.instructions:
                si = ins.sync_info
                if si is not None and len(si.on_wait) > cap:
                    waits = list(si.on_wait)
                    extra, keep = waits[:-cap], waits[-cap:]
                    for w in extra:
                        ev = mybir.InstEventSemaphore(
                            name=f"waitsplit_{n_new}", ins=[], outs=[]
                        )
                        ev.engine = ins.engine
                        ev.sync_info = mybir.SyncInfo(on_wait=[w], on_update=[])
                        new_list.append(ev)
                        n_new += 1
                    ins.sync_info = mybir.SyncInfo(
                        on_wait=keep, on_update=list(si.on_update)
                    )
                new_list.append(ins)
            b.instructions[:] = new_list
    return n_new


def _drain_banks(s):
    """PSUM bank range whose l >= s, rounded down to whole banks (recomputing
    p at l < s is a no-op; partial-partition PSUM APs spanning > 32
    partitions are rejected by the BIR verifier)."""
    if s < 128:
        return slice(0, 4)
    if s == 128:
        return slice(1, 4)
    return slice(2, 4)


def _build_hybrid_program(reps=1, legalize=True):
    from contextlib import ExitStack

    import concourse.bass as bass
    import concourse.mybir as mybir
    import concourse.tile as tile

    f16 = mybir.dt.float16
    f32 = mybir.dt.float32
    u8 = mybir.dt.uint8

    nc = bass.Bass(target_bir_lowering=False, debug=False)
    xT_ext = nc.declare_dram_parameter("xT", [L, PE_ROWS], f16, isOutput=False)
    mT_ext = nc.declare_dram_parameter("mT", [L, PE_ROWS], u8, isOutput=False)
    yT_ext = nc.declare_dram_parameter("yT", [L, PE_ROWS], f16, isOutput=True)
    mats_ext = nc.declare_dram_parameter("mats", [128, N_EYES, 128], f16, isOutput=False)
    xf_ext = nc.declare_dram_parameter("xf", [FLAT_ROWS, L], f16, isOutput=False)
    mf_ext = nc.declare_dram_parameter("mf", [FLAT_ROWS, L], u8, isOutput=False)
    yf_ext = nc.declare_dram_parameter("yf", [FLAT_ROWS, L], f16, isOutput=True)

    with tile.TileContext(nc) as tc, ExitStack() as ctx:
        cp = ctx.enter_context(tc.tile_pool(name="cp", bufs=1))
        pp = ctx.enter_context(tc.tile_pool(name="pp", bufs=3))
        m8p = ctx.enter_context(tc.tile_pool(name="m8p", bufs=3))
        mpp = ctx.enter_context(tc.tile_pool(name="mpp", bufs=3))
        dp = ctx.enter_context(tc.tile_pool(name="dp", bufs=4))
        wp = ctx.enter_context(tc.tile_pool(name="wp", bufs=2, space="PSUM"))
        xfp = ctx.enter_context(tc.tile_pool(name="xfp", bufs=3))
        mf8p = ctx.enter_context(tc.tile_pool(name="mf8p", bufs=3))
        mfp = ctx.enter_context(tc.tile_pool(name="mfp", bufs=3))
        tfp = ctx.enter_context(tc.tile_pool(name="tfp", bufs=2))

        mats_t = cp.tile([128, N_EYES, 128], f16)
        nc.sync.dma_start(mats_t[:], mats_ext[:])

        def eye(k):
            return mats_t[:, EYE_IDX[k], :]

        def flat_ops():
            """Generator: yields after each schedulable unit of flat work."""
            for r in range(N_FLAT_TILES):
                rows = slice(r * FLAT_TILE_ROWS, (r + 1) * FLAT_TILE_ROWS)
                xt = xfp.tile([128, GF, L], f16)
                m8 = mf8p.tile([128, GF, L], u8)
                mt = mfp.tile([128, GF, L], f16)
                tt = tfp.tile([128, GF, L], f16)
                nc.sync.dma_start(
                    xt[:], xf_ext[rows, :].rearrange("(g p) l -> p g l", p=128)
                )
                nc.sync.dma_start(
                    m8[:], mf_ext[rows, :].rearrange("(g p) l -> p g l", p=128)
                )
                nc.scalar.copy(mt[:], m8[:])
                yield
                for s in SHIFTS:
                    nc.vector.tensor_mul(
                        tt[:, :, s:], xt[:, :, : L - s], mt[:, :, s:]
                    )
                    nc.vector.tensor_add(
                        xt[:, :, s:], xt[:, :, s:], tt[:, :, s:]
                    )
                    yield
                nc.sync.dma_start(
                    yf_ext[rows, :].rearrange("(g p) l -> p g l", p=128), xt[:]
                )
                yield

        def emit_mms(wt, pt, s, is_last):
            """shift-s accumulate: w += shift_s(p). One matmul per PSUM bank
            (a single matmul's moving free size is capped at 512 = 1 bank)."""
            if s < 128:
                for b in range(4):
                    nc.tensor.matmul(
                        wt[:, b, :], eye(s), pt[:, b, :],
                        start=False, stop=False, skip_group_check=True,
                    )
                for b in range(1, 4):
                    nc.tensor.matmul(
                        wt[:, b, :], eye(s - 128), pt[:, b - 1, :],
                        start=False, stop=False, skip_group_check=True,
                    )
            elif s == 128:
                for b in range(1, 4):
                    nc.tensor.matmul(
                        wt[:, b, :], eye(0), pt[:, b - 1, :],
                        start=False, stop=False, skip_group_check=True,
                    )
            else:
                for b in range(2, 4):
                    nc.tensor.matmul(
                        wt[:, b, :], eye(0), pt[:, b - 2, :],
                        start=False, stop=is_last, skip_group_check=True,
                    )

        def body(_iv=None):
            fgen = flat_ops()
            n_units = N_FLAT_TILES * (len(SHIFTS) + 2)
            n_slots = (N_SUP_PE // 2) * (len(SHIFTS) + 1)
            credit, fdone = 0.0, False
            rate = n_units / n_slots

            def pump(scale=1.0):
                nonlocal credit, fdone
                credit += rate * scale
                while credit >= 1.0 and not fdone:
                    credit -= 1.0
                    try:
                        next(fgen)
                    except StopIteration:
                        fdone = True

            # Pre-roll a couple of filler units so the DVE has work during
            # the first pair's DMA/matmul warmup.
            for _ in range(2):
                try:
                    next(fgen)
                except StopIteration:
                    fdone = True

            n_pairs = N_SUP_PE // 2
            for jp in range(n_pairs):
                # The DVE executes queued filler early whenever PE-chain muls
                # stall, so uniform pacing runs the filler dry before the last
                # pairs. Skew emission toward the tail.
                pace = 0.55 if jp < n_pairs // 2 else 1.45
                pair = (2 * jp, 2 * jp + 1)
                pts, mts, wts = [], [], []
                for j in pair:
                    cols = slice(j * NB, (j + 1) * NB)
                    pt = pp.tile([128, 4, NB], f16)  # p_k, updated in place
                    m8t = m8p.tile([128, 4, NB], u8)
                    mt = mpp.tile([128, 4, NB], f16)
                    nc.sync.dma_start(
                        pt[:],
                        xT_ext[:, cols].rearrange("(blk p) b -> p blk b", p=128),
                    )
                    nc.sync.dma_start(
                        m8t[:],
                        mT_ext[:, cols].rearrange("(blk p) b -> p blk b", p=128),
                    )
                    nc.scalar.copy(mt[:], m8t[:])
                    pts.append(pt)
                    mts.append(mt)
                for i in range(2):
                    wt = wp.tile([128, 4, NB], f32)
                    for b in range(4):
                        nc.tensor.matmul(
                            wt[:, b, :], eye(0), pts[i][:, b, :],
                            start=True, stop=False, skip_group_check=True,
                        )
                    wts.append(wt)
                pump(pace)
                for s in SHIFTS:
                    is_last = s == SHIFTS[-1]
                    for i in range(2):
                        emit_mms(wts[i], pts[i], s, is_last)
                    dts = []
                    for i in range(2):
                        dt = dp.tile([128, 4, NB], f16)
                        bs = _drain_banks(s)
                        nc.scalar.copy(dt[:, bs, :], wts[i][:, bs, :])
                        dts.append(dt)
                    for i in range(2):
                        bs = _drain_banks(s)
                        nc.vector.tensor_mul(
                            pts[i][:, bs, :], dts[i][:, bs, :], mts[i][:, bs, :]
                        )
                    pump(pace)
                for i, j in enumerate(pair):
                    cols = slice(j * NB, (j + 1) * NB)
                    nc.sync.dma_start(
                        yT_ext[:, cols].rearrange("(blk p) b -> p blk b", p=128),
                        pts[i][:],
                    )

            while not fdone:
                try:
                    next(fgen)
                except StopIteration:
                    fdone = True

        if reps == 1:
            body()
        else:
            with tc.For_i(0, reps, 1) as iv:
                body(iv)

    if legalize:
        _legalize_waits(nc)
    return nc


_cached = {}


def kernel(x, mask):
    global _last_results
    from concourse.bass_utils import run_bass_kernel_spmd

    x = np.asarray(x)
    m = np.asarray(mask)
    assert x.shape == (B, L) and m.shape == (B, L)
    # Host pre-masking: p0 = where(mask, x, 0)
    x16 = np.where(m, x, np.float32(0.0)).astype(np.float16)
    m8 = m.astype(np.uint8)

    if "hybrid" not in _cached:
        _cached["hybrid"] = _build_hybrid_program()
    nc = _cached["hybrid"]

    core_ids = list(range(N_CORES))
    mats = _eye_mats()
    in_maps = []
    for i in core_ids:
        xc = x16[i * BP : (i + 1) * BP]
        mc = m8[i * BP : (i + 1) * BP]
        in_maps.append(
            {
                "xf": np.ascontiguousarray(xc[:FLAT_ROWS]),
                "mf": np.ascontiguousarray(mc[:FLAT_ROWS]),
                "xT": np.ascontiguousarray(xc[FLAT_ROWS:].T),
                "mT": np.ascontiguousarray(mc[FLAT_ROWS:].T),
                "mats": mats,
            }
        )

    res = run_bass_kernel_spmd(nc, in_maps, core_ids)
    _last_results = res

    out = np.empty((B, L), dtype=np.float32)
    for i in core_ids:
        out[i * BP : i * BP + FLAT_ROWS] = res.results[i]["yf"].astype(np.float32)
        out[i * BP + FLAT_ROWS : (i + 1) * BP] = (
            res.results[i]["yT"].T.astype(np.float32)
        )
    return out
